# revision 6
# baseline (speedup 1.0000x reference)
"""Causal MHA with RoPE on 8 Trainium2 NeuronCores.

Sharding: core c -> batch b = c//2, head-group g = c%2 (8 heads each).
Per core: qkv projection (fp32r matmuls), RoPE (DVE), causal attention
(scores^T layout [k, q] -> no transposes; softmax without max-subtraction,
denominator via ones-row appended to V), output projection (partial over
the core's heads). Host sums the two head-group partials per batch + bias.

Hardcoded problem: B=4, T=2048, C=1024, H=16, hs=64.
"""
import math
import numpy as np
import contextlib

import concourse.bass as bass
import concourse.tile as tile
from concourse import bacc, mybir
from concourse.bass_utils import run_bass_kernel_spmd

B, T, C = 4, 2048, 1024
H, HS = 16, 64
HG = 8            # heads per core
N_CORES = 8
NQ = 512          # q-super width
NKT = T // 128    # 16 k-tiles
NJS = T // NQ     # 4 q-supers

f32 = mybir.dt.float32
f32r = mybir.dt.float32r
EXP = mybir.ActivationFunctionType.Exp

_NC_CACHE = {}
LEVEL = 3  # 1=qkv only, 2=+attention, 3=full(norm+proj)


def build_nc(iters: int = 1):
    key = (iters, LEVEL)
    if key in _NC_CACHE:
        return _NC_CACHE[key]
    nc = bacc.Bacc("TRN2", target_bir_lowering=False, debug=False,
                   num_devices=N_CORES)

    xt_ap = nc.dram_tensor("xt", [C, T], f32r, kind="ExternalInput").ap()
    wqkv_ap = nc.dram_tensor("wqkv", [C, 3, 512], f32r, kind="ExternalInput").ap()
    wp_ap = nc.dram_tensor("wp", [512, C], f32r, kind="ExternalInput").ap()
    cos_ap = nc.dram_tensor("cost", [128, T], f32, kind="ExternalInput").ap()
    sin_ap = nc.dram_tensor("sint", [128, T], f32, kind="ExternalInput").ap()
    msk_ap = nc.dram_tensor("trimask", [128, 4, NQ], f32, kind="ExternalInput").ap()
    one_ap = nc.dram_tensor("vones", [128, NKT, HG, 1], f32r, kind="ExternalInput").ap()
    bones_ap = nc.dram_tensor("bones", [1, 64], f32r, kind="ExternalInput").ap()
    out_ap = nc.dram_tensor("outT", [C, T], f32, kind="ExternalOutput").ap()

    with tile.TileContext(nc) as tc, contextlib.ExitStack() as ctx:
        pq = ctx.enter_context(tc.tile_pool(name="pq", bufs=1))
        pk = ctx.enter_context(tc.tile_pool(name="pk", bufs=1))
        pv = ctx.enter_context(tc.tile_pool(name="pv", bufs=1))
        pxt = ctx.enter_context(tc.tile_pool(name="pxt", bufs=1))
        pw = ctx.enter_context(tc.tile_pool(name="pw", bufs=3))
        ptab = ctx.enter_context(tc.tile_pool(name="ptab", bufs=2))
        pscr = ctx.enter_context(tc.tile_pool(name="pscr", bufs=2))
        pwp = ctx.enter_context(tc.tile_pool(name="pwp", bufs=1))
        pexp = ctx.enter_context(tc.tile_pool(name="pexp", bufs=4))
        pyt = ctx.enter_context(tc.tile_pool(name="pyt", bufs=2))
        pmask = ctx.enter_context(tc.tile_pool(name="pmask", bufs=1))
        prc = ctx.enter_context(tc.tile_pool(name="prc", bufs=2))
        pbc = ctx.enter_context(tc.tile_pool(name="pbc", bufs=2))
        pout = ctx.enter_context(tc.tile_pool(name="pout", bufs=3))
        pps = ctx.enter_context(tc.tile_pool(name="pps", bufs=8, space="PSUM"))

        # constants loaded once (outside the timing loop)
        wp_t = pwp.tile([128, 4, C], f32r, tag="wp")
        nc.sync.dma_start(out=wp_t, in_=wp_ap.rearrange("(kt p) e -> p kt e", p=128))
        mask_t = pmask.tile([128, 4, NQ], f32, tag="mask")
        nc.sync.dma_start(out=mask_t, in_=msk_ap)
        bones_t = pmask.tile([1, 64], f32r, tag="bones")
        nc.sync.dma_start(out=bones_t, in_=bones_ap)

        def body(_iv):
            # persistent-per-iteration tensors
            qt_t = pq.tile([128, 4, T], f32r, tag="qt")
            kt_t = pk.tile([128, 4, T], f32r, tag="kt")
            v_t = pv.tile([128, NKT, HG, HS + 1], f32r, tag="vt")
            nc.sync.dma_start(out=v_t[:, :, :, HS:HS + 1], in_=one_ap)

            # ---- phase 1: qkv + RoPE ----
            for ts in range(NJS):
                tsl = slice(ts * NQ, (ts + 1) * NQ)
                xt_t = pxt.tile([128, 8, NQ], f32r, tag="xt")
                nc.sync.dma_start(
                    out=xt_t, in_=xt_ap[:, tsl].rearrange("(kt p) n -> p kt n", p=128))
                cos_t = ptab.tile([128, NQ], f32, tag="cos")
                nc.sync.dma_start(out=cos_t, in_=cos_ap[:, tsl])
                sin_t = ptab.tile([128, NQ], f32, tag="sin")
                nc.sync.dma_start(out=sin_t, in_=sin_ap[:, tsl])

                for s, dst in ((0, qt_t), (1, kt_t)):
                    pss = []
                    for m in range(4):
                        ps_m = pps.tile([128, NQ], f32, tag="ps", name=f"qk{ts}{s}{m}")
                        pss.append(ps_m)
                    for kt in range(8):
                        w_t = pw.tile([128, 512], f32r, tag="w")
                        nc.sync.dma_start(
                            out=w_t, in_=wqkv_ap[kt * 128:(kt + 1) * 128, s, :])
                        for m in range(4):
                            nc.tensor.matmul(
                                pss[m][:], w_t[:, m * 128:(m + 1) * 128],
                                xt_t[:, kt, :], start=(kt == 0), stop=(kt == 7))
                    # RoPE pairs (even m, odd m+2)
                    for pi in range(2):
                        e, o = pss[pi], pss[pi + 2]
                        de = dst[:, pi, tsl]
                        do = dst[:, pi + 2, tsl]
                        t1 = pscr.tile([128, NQ], f32, tag="scr")
                        nc.vector.tensor_mul(de, e[:], cos_t[:])
                        nc.vector.tensor_mul(t1[:], o[:], sin_t[:])
                        nc.vector.tensor_sub(de, de, t1[:])
                        t2 = pscr.tile([128, NQ], f32, tag="scr")
                        nc.vector.tensor_mul(do, e[:], sin_t[:])
                        nc.vector.tensor_mul(t2[:], o[:], cos_t[:])
                        nc.vector.tensor_add(do, do, t2[:])

                # v sweep (natural layout [tok, hd])
                vps = []
                for tt in range(4):
                    ps_v = pps.tile([128, NQ], f32, tag="ps", name=f"v{ts}{tt}")
                    vps.append(ps_v)
                for kt in range(8):
                    wv_t = pw.tile([128, 512], f32r, tag="w")
                    nc.sync.dma_start(
                        out=wv_t, in_=wqkv_ap[kt * 128:(kt + 1) * 128, 2, :])
                    for tt in range(4):
                        nc.tensor.matmul(
                            vps[tt][:], xt_t[:, kt, tt * 128:(tt + 1) * 128],
                            wv_t[:], start=(kt == 0), stop=(kt == 7))
                for tt in range(4):
                    nc.vector.tensor_copy(
                        v_t[:, ts * 4 + tt, :, 0:HS], vps[tt][:])

            if LEVEL < 2:
                ob0 = pout.tile([128, NQ], f32, tag="ob")
                nc.vector.tensor_copy(ob0[:], qt_t[:, 0, 0:NQ])
                nc.sync.dma_start(out=out_ap[0:128, 0:NQ], in_=ob0[:])
                return
            # ---- phase 2: attention + proj ----
            for j in range(NJS):
                jsl = slice(j * NQ, (j + 1) * NQ)
                yt_t = pyt.tile([128, 4, NQ], f32r, tag="yt")
                for hl in range(HG):
                    e_i, o_i = hl // 4, 2 + hl // 4
                    po = 32 * (hl % 4)
                    pv_ps = pps.tile([HS + 1, NQ], f32, tag="ps")
                    nk = 4 * j + 4
                    for i in range(nk):
                        sc = pps.tile([128, NQ], f32, tag="ps")
                        isl = slice(i * 128, (i + 1) * 128)
                        nc.tensor.matmul(
                            sc[:], kt_t[po:po + 32, e_i, isl],
                            qt_t[po:po + 32, e_i, jsl],
                            start=True, stop=False, tile_position=(po, 0))
                        nc.tensor.matmul(
                            sc[:], kt_t[po:po + 32, o_i, isl],
                            qt_t[po:po + 32, o_i, jsl],
                            start=False, stop=True, tile_position=(po, 0))
                        r = i - 4 * j
                        if r >= 0:
                            nc.vector.tensor_add(sc[:], sc[:], mask_t[:, r, :])
                        ex = pexp.tile([128, NQ], f32r, tag="exp")
                        nc.scalar.activation(ex[:], sc[:], EXP)
                        nc.tensor.matmul(
                            pv_ps[:], v_t[:, i, hl, :], ex[:],
                            start=(i == 0), stop=(i == nk - 1))
                    if LEVEL < 3:
                        nc.vector.tensor_copy(
                            yt_t[(hl % 2) * 64:(hl % 2) * 64 + 64, hl // 2, :],
                            pv_ps[0:HS, :])
                        continue
                    rc = prc.tile([1, NQ], f32r, tag="rc")
                    with nc.allow_low_precision(reason="f32r is 32-bit"):
                        nc.vector.reciprocal(rc[:], pv_ps[HS:HS + 1, :])
                    bcp = pps.tile([64, NQ], f32, tag="ps", name=f"bc{j}{hl}")
                    nc.tensor.matmul(bcp[:], bones_t[:], rc[:],
                                     start=True, stop=True)
                    bc = pbc.tile([64, NQ], f32, tag="bc")
                    nc.vector.tensor_copy(bc[:], bcp[:])
                    nc.vector.tensor_mul(
                        yt_t[(hl % 2) * 64:(hl % 2) * 64 + 64, hl // 2, :],
                        pv_ps[0:HS, :], bc[:])
                # proj for this q-super
                for m in range(8):
                    pj = pps.tile([128, NQ], f32, tag="ps")
                    for kt in range(4):
                        nc.tensor.matmul(
                            pj[:], wp_t[:, kt, m * 128:(m + 1) * 128],
                            yt_t[:, kt, :], start=(kt == 0), stop=(kt == 3))
                    ob = pout.tile([128, NQ], f32, tag="ob")
                    nc.vector.tensor_copy(ob[:], pj[:])
                    nc.sync.dma_start(
                        out=out_ap[m * 128:(m + 1) * 128, jsl], in_=ob[:])

        if iters == 1:
            body(0)
        else:
            with tc.For_i(0, iters) as iv:
                body(iv)

    nc.compile()
    _NC_CACHE[key] = nc
    return nc


def make_in_maps(x, W_qkv, W_proj):
    """Per-core host-side sharding + RoPE-layout permutation."""
    # x1-first column permutation within a head-group (8 heads x 64 dims):
    # [h0 evens, h1 evens, ..., h7 evens, h0 odds, ..., h7 odds]
    perm = []
    for parity in (0, 1):
        for hlc in range(HG):
            perm.extend(hlc * HS + d for d in range(parity, HS, 2))
    perm = np.asarray(perm)

    pos = np.arange(T, dtype=np.float64)
    inv_freq = 1.0 / (10000.0 ** (np.arange(0, HS, 2, dtype=np.float64) / HS))
    freqs = pos[:, None] * inv_freq[None, :]          # (T, 32)
    cost = np.tile(np.cos(freqs).T, (4, 1)).astype(np.float32)   # (128, T)
    sint = np.tile(np.sin(freqs).T, (4, 1)).astype(np.float32)

    kk = np.arange(128)[:, None]
    qq = np.arange(NQ)[None, :]
    trimask = np.stack(
        [np.where(128 * r + kk <= qq, 0.0, -30000.0) for r in range(4)],
        axis=1).astype(np.float32)                     # (128, 4, NQ)
    vones = np.ones((128, NKT, HG, 1), np.float32)

    scale = 1.0 / math.sqrt(HS)
    in_maps = []
    for c in range(N_CORES):
        b, g = c // 2, c % 2
        base = g * HG * HS
        wq = W_qkv[:, base + perm] * scale
        wk = W_qkv[:, C + base + perm]
        wv = W_qkv[:, 2 * C + base: 2 * C + base + HG * HS]
        wqkv = np.stack([wq, wk, wv], axis=1).astype(np.float32)  # (C, 3, 512)
        in_maps.append({
            "xt": np.ascontiguousarray(x[b].T).astype(np.float32),
            "wqkv": np.ascontiguousarray(wqkv),
            "wp": np.ascontiguousarray(W_proj[base:base + HG * HS, :]).astype(np.float32),
            "cost": cost, "sint": sint, "trimask": trimask, "vones": vones,
            "bones": np.ones((1, 64), np.float32),
        })
    return in_maps


def kernel(x, W_qkv, W_proj, b_proj):
    x = np.asarray(x); W_qkv = np.asarray(W_qkv)
    W_proj = np.asarray(W_proj); b_proj = np.asarray(b_proj)
    nc = build_nc(1)
    in_maps = make_in_maps(x, W_qkv, W_proj)
    res = run_bass_kernel_spmd(nc, in_maps, list(range(N_CORES)))
    out = np.empty((B, T, C), np.float32)
    for b in range(B):
        acc = res.results[2 * b]["outT"] + res.results[2 * b + 1]["outT"]
        out[b] = acc.T + b_proj[None, :]
    return out
